# revision 5
# baseline (speedup 1.0000x reference)
"""Anti-diagonal wavefront variant.

Sites on diagonal d = r+c depend only on diagonal d-1 (both the up and left
neighbors), so each diagonal is a pure elementwise update -- no serial scan:

    t[j] = max(x_prev[up_j], x_prev[left_j])     (DVE tensor_tensor, bf16)
    x[j] = c[j] is_gt t[j]                       (DVE tensor_tensor, bf16)

bf16 tensor_tensor streams at ~0.59 ns/elem (DVE 2x packed mode) vs the
row-scan's ~2.15 ns/elem, so the wavefront's 2 ops/site beat the scan's
fused row update by ~1.8x.

Layout (per partition): diagonal-major; each diagonal is [g][len_d+1] (one
leading pad slot per group) plus ONE extra slot after the last group.  Pads
carry compare value -1 (from dummy byte 0x7F), which forces x = 0 there, so
pads are *computed*, never memset; the extra end slot doubles as group 63's
missing trailing pad for the growing-phase window over-read.  Every
diagonal (pads included) is one contiguous run for DMA.

All DVE output APs are even-based, gap-free, full-range: odd-offset or
gapped output APs mis-write in the DVE 2x packed mode (found the hard way).

x is bf16 (0.0/1.0); the host converts to int32.  Tiny head (d < DH) and
tail (d >= DT) diagonals accumulate in persistent buffers and ship as one
DMA each; mid diagonals rotate through a 12-deep ring, DMA'd per diagonal.
"""

import numpy as np

import concourse.bacc as bacc
import concourse.mybir as mybir
from concourse.tile import TileContext
from concourse.bass_utils import run_bass_kernel_spmd

N_CORES = 8
S_TOTAL = 65536
R = 32
C = 32
P = 128

SPC = S_TOTAL // N_CORES  # 8192
G = SPC // P  # 64

ND = R + C - 1  # 63 diagonals
LENS = [min(d, R - 1) - max(0, d - (C - 1)) + 1 for d in range(ND)]
PLENS = [l + 1 for l in LENS]  # leading pad per group
NELEM = [G * pl + 1 for pl in PLENS]  # + one shared end pad slot
DOFF = np.concatenate([[0], np.cumsum(NELEM)])
TOT = int(DOFF[-1])  # 69631 elems per partition

DH = 10  # diags [0, DH) live in the persistent head buffer, one DMA
DT = 52  # diags [DT, ND) live in the persistent tail buffer, one DMA
NRING = 12
NT = 6

F32 = mybir.dt.float32
BF16 = mybir.dt.bfloat16
I8 = mybir.dt.int8
ALU = mybir.AluOpType


def _make_slabs():
    budgets = [12, 24, 40, 70, 110]
    slabs = []
    d = 0
    bi = 0
    while d < ND:
        budget = budgets[bi] if bi < len(budgets) else 150
        bi += 1
        e = d
        tot = 0
        while e < ND and (tot == 0 or tot + NELEM[e] <= budget * G):
            tot += NELEM[e]
            e += 1
        slabs.append((d, e))
        d = e
    return slabs


SLABS = _make_slabs()
MAXSLAB = max(int(DOFF[e] - DOFF[s]) for s, e in SLABS)
M = 8  # front margin: growing-phase windows read from base-1
XBUF = M + G * max(PLENS) + 1 + 7
HBUF = M + int(DOFF[DH])  # head buffer: diags [0, DH) at DOFF offsets
TBUF = M + int(DOFF[ND] - DOFF[DT])


def _win(tile, base, sp, n):
    """Strided view [P][G: stride sp][n: stride 1] at elem offset base."""
    v = tile[:, base : base + (G - 1) * sp + n].rearrange("p (a b) -> p a b", b=1)
    ap = v.ap
    ap[1] = [sp, G]
    ap[2] = [1, n]
    v.ap = ap
    return v


def build_nc():
    nc = bacc.Bacc("TRN2", target_bir_lowering=False, debug=False)
    u = nc.declare_dram_parameter("u", [P, TOT], I8, isOutput=False)
    cfg = nc.declare_dram_parameter("config", [P, TOT], BF16, isOutput=True)

    with TileContext(nc) as tc:
        with (
            tc.tile_pool(name="const", bufs=1) as constp,
            tc.tile_pool(name="b", bufs=3) as bp,
            tc.tile_pool(name="c", bufs=3) as cp,
            tc.tile_pool(name="x", bufs=NRING) as xp,
            tc.tile_pool(name="t", bufs=NT) as tp,
            tc.tile_pool(name="ht", bufs=2) as htp,
        ):
            thr = constp.tile([P, 1], F32, tag="thr")
            pre_i = constp.tile([P, 1], I8, tag="pre_i")
            pre_o = constp.tile([P, 1], BF16, tag="pre_o")
            nc.gpsimd.memset(thr[:], 62.5)
            nc.gpsimd.memset(pre_i[:], 0)
            # Preload the Sign activation table during the preamble so the
            # first real compare doesn't pay ACT_TABLE_LOAD (~1.3 us).
            nc.scalar.activation(
                out=pre_o[:],
                in_=pre_i[:],
                func=mybir.ActivationFunctionType.Sign,
                bias=thr[:],
                scale=-1.0,
            )

            xb = [xp.tile([P, XBUF], BF16, tag="x", name=f"x{i}") for i in range(NRING)]
            tb = [tp.tile([P, XBUF], BF16, tag="t", name=f"t{i}") for i in range(NT)]
            hbuf = htp.tile([P, HBUF], BF16, tag="hbuf")
            tbuf = htp.tile([P, TBUF], BF16, tag="tbuf")
            # t tiles must start at 0: each is_gt reads one slot past the
            # max's write range (the shared end pad), and x values written
            # there must be 0.  After that, t only ever holds {0,1}.
            # Ordered by first use (tb[1] consumed first, at diag 1).
            for i in [1, 2, 3, 4, 5, 0]:
                nc.gpsimd.memset(tb[i][:], 0)

            def xaddr(d):
                if d < DH:
                    return hbuf, M + int(DOFF[d])
                if d >= DT:
                    return tbuf, M + int(DOFF[d] - DOFF[DT])
                return xb[d % NRING], M

            ctiles = {}

            def issue_slab(si):
                s, e = SLABS[si]
                nelem = int(DOFF[e] - DOFF[s])
                b = bp.tile([P, MAXSLAB], I8, tag="b")
                nc.sync.dma_start(
                    out=b[:, 0:nelem], in_=u[:, int(DOFF[s]) : int(DOFF[e])]
                )
                c = cp.tile([P, MAXSLAB], BF16, tag="c")
                # c = sign(62.5 - byte) in {-1,+1}: +1 iff u < 0.5; pads
                # (0x7F) become -1, forcing x = 0 at pad slots.
                nc.scalar.activation(
                    out=c[:, 0:nelem],
                    in_=b[:, 0:nelem],
                    func=mybir.ActivationFunctionType.Sign,
                    bias=thr[:],
                    scale=-1.0,
                )
                for d in range(s, e):
                    ctiles[d] = (c, int(DOFF[d] - DOFF[s]))

            issue_slab(0)
            issue_slab(1)
            issue_slab(2)
            next_slab = 3

            for d in range(ND):
                if next_slab < len(SLABS) and d >= SLABS[next_slab - 2][0]:
                    issue_slab(next_slab)
                    next_slab += 1

                pl = PLENS[d]
                n1 = NELEM[d]  # G*pl + 1
                c, coff = ctiles[d]
                cview = c[:, coff : coff + n1]
                xt, xo = xaddr(d)
                if d == 0:
                    # x(0,0) = bern: c is_gt 0 (pads -1 -> 0)
                    nc.vector.tensor_scalar(
                        out=xt[:, xo : xo + n1],
                        in0=cview,
                        scalar1=0.0,
                        scalar2=None,
                        op0=ALU.is_gt,
                    )
                else:
                    pt, po = xaddr(d - 1)
                    sp = PLENS[d - 1]
                    t = tb[d % NT]
                    # growing (d < R): t[j] = max(Xp[j-1], Xp[j]); j=0 reads
                    # the margin (garbage) but j=0 is a pad slot (c = -1).
                    # shrinking (d >= R): t[j] = max(Xp[j], Xp[j+1]).
                    # Full-range even-based gap-free writes only.
                    o = -1 if d <= R - 1 else 0
                    nc.vector.tensor_tensor(
                        out=t[:, 0 : G * pl].rearrange("p (g j) -> p g j", j=pl),
                        in0=_win(pt, po + o, sp, pl),
                        in1=_win(pt, po + o + 1, sp, pl),
                        op=ALU.max,
                    )
                    # x = c > t ; pad slots have c = -1 -> 0 regardless of t.
                    # The extra end slot reads stale t (harmless: c = -1).
                    nc.vector.tensor_tensor(
                        out=xt[:, xo : xo + n1],
                        in0=cview,
                        in1=t[:, 0:n1],
                        op=ALU.is_gt,
                    )
                if d == DH + 1:
                    # head buffer final: one DMA for diags [0, DH)
                    nc.sync.dma_start(
                        out=cfg[:, 0 : int(DOFF[DH])],
                        in_=hbuf[:, M : M + int(DOFF[DH])],
                    )
                if DH <= d < DT:
                    nc.sync.dma_start(
                        out=cfg[:, int(DOFF[d]) : int(DOFF[d]) + n1],
                        in_=xt[:, xo : xo + n1],
                    )
                # tail buffer ships in pieces as its diagonals finish, so
                # only the last sliver remains after the final compute
                if d in (56, 60):
                    lo = DT if d == 56 else 56
                    nc.sync.dma_start(
                        out=cfg[:, int(DOFF[lo]) : int(DOFF[d])],
                        in_=tbuf[:, M + int(DOFF[lo] - DOFF[DT]) :
                                 M + int(DOFF[d] - DOFF[DT])],
                    )
            nc.sync.dma_start(
                out=cfg[:, int(DOFF[60]) : int(DOFF[ND])],
                in_=tbuf[:, M + int(DOFF[60] - DOFF[DT]) :
                         M + int(DOFF[ND] - DOFF[DT])],
            )
    nc.compile()
    return nc


# Static index map: flat diag-major position -> site index (g*1024 + r*C+c),
# -1 = pad
def _build_idx():
    idx = np.full(TOT, -1, np.int64)
    for d in range(ND):
        r0 = max(0, d - (C - 1))
        for g in range(G):
            base = int(DOFF[d]) + g * PLENS[d]
            for i in range(LENS[d]):
                r = r0 + i
                c = d - r
                idx[base + 1 + i] = g * (R * C) + r * C + c
    return idx


IDX = _build_idx()
VALID = IDX >= 0
IDXC = np.where(VALID, IDX, 0)


def host_permute_u(u_core):
    """[spc, 32, 32] f32 -> diag-major top-byte plane [P, TOT] int8."""
    b3 = u_core.reshape(-1).view(np.uint8)[3::4]
    v = b3.reshape(G, P, R * C).transpose(1, 0, 2).reshape(P, G * R * C)
    out = v[:, IDXC]
    out[:, ~VALID] = 0x7F
    return out.view(np.int8)


def host_unpermute_cfg(res):
    """{config: [P, TOT] bf16} -> [spc, 32, 32] int32."""
    v = np.asarray(res["config"]).view(np.uint16)
    bits = (v != 0)
    out = np.zeros((P, G * R * C), np.int32)
    out[:, IDX[VALID]] = bits[:, VALID]
    # sample s = g*P + p
    return out.reshape(P, G, R * C).transpose(1, 0, 2).reshape(SPC, R, C)


_NC_CACHE = {}


def _get_nc():
    if "nc" not in _NC_CACHE:
        _NC_CACHE["nc"] = build_nc()
    return _NC_CACHE["nc"]


def kernel(u, n_rows=32, n_cols=32, **_):
    u = np.ascontiguousarray(np.asarray(u), dtype=np.float32)
    assert u.shape == (S_TOTAL, R, C), u.shape
    assert int(n_rows) == R and int(n_cols) == C

    nc = _get_nc()
    in_maps = [
        {"u": host_permute_u(u[i * SPC : (i + 1) * SPC])} for i in range(N_CORES)
    ]
    res = run_bass_kernel_spmd(nc, in_maps, list(range(N_CORES)))
    out = np.concatenate(
        [host_unpermute_cfg(res.results[i]) for i in range(N_CORES)], axis=0
    )
    return out.reshape(S_TOTAL, R, C)
